# revision 59
# baseline (speedup 1.0000x reference)
"""Trainium2 Bass kernel for cross-covariance multi-head attention (XCA).

Reference computation (per batch b of 8, all fp32):
    q = l2norm_tokens((x @ Wq.T) -> [h, d, n])   # norm over n (tokens)
    k = l2norm_tokens((x @ Wk.T) -> [h, d, n])
    v = (x @ Wv.T) -> [h, d, n]
    attn = softmax(k @ q^T * scale_h, axis=-1)   # [h, d, d], contraction over n
    out = attn @ v                               # [h, d, n]
    y = raw_view(out, [n, c]) @ Wo.T + bo        # scrambled channel/token view

Sharding: data-parallel over batch, one batch element per NeuronCore (8 cores).

Gram-matrix restructuring (per core, C=1024, T=4096, P=128):
  All of phase 1 only needs G = X^T X  [C, C]:
    A0_raw       = Wk G Wq^T   (per-head diagonal blocks)
    ||Kraw_d||^2 = diag(Wk G Wk^T),  ||Qraw_e||^2 = diag(Wq G Wq^T)
  computed as GWk = G Wk^T, GWq = G Wq^T then small contractions. G (upper
  triangle only; lower restored by PE transposes), GWk, GWq run in fp8
  (e4m3) with DoubleRow 2x matmul throughput - the logit path tolerates
  fp8 because errors average over 1024-term quadratic forms. Norm
  products are taken straight off the GW psum (scale G*W/2); the 8x
  excess on ss cancels against host-side scb*8.
  Phase 2 folds attention into M^T = Wv^T blockdiag(P^T) and uses the raw
  view structure: output rows t=4*row+chunk are y[t,:] = M[row] Z_chunk:
    Z_chunk [C, C] = X[chunk tokens]^T @ Wo^T    (4 chunks of 1024 tokens)
    Y_chunk [C, C] = (M^T)^T @ Z_chunk -> strided DMA to y rows.
"""
import sys

for _p in ("/opt/trn_rl_repo",):
    if _p not in sys.path:
        sys.path.insert(0, _p)

from contextlib import ExitStack

import numpy as np

import concourse.bass as bass
import concourse.mybir as mybir
import concourse.tile as tile
from concourse import bacc
from concourse.masks import make_identity

f32 = mybir.dt.float32
f32r = mybir.dt.float32r
bf16 = mybir.dt.bfloat16
f8 = mybir.dt.float8e4
P = 128
N_CORES = 8
H_FULL = 16
C_FULL = 1024
T_FULL = 4096
EPS = 1e-12
G_SCALE = 1.0 / 32.0   # G psum -> fp8 scale (|G| <= ~4600 -> <= 144 < 240)
W_SCALE = 16.0         # host premultiplies wq/wk by this before fp8 cast
GW_SCALE = 1.0 / 8.0   # GWk psum -> fp8; net gwk = G*Wk^T/16
DR = mybir.MatmulPerfMode.DoubleRow
N_WARM = 30            # HAM warmup matmuls during the initial DMA wait


def emit_kernel(tc, handles, C, T):
    nc = tc.nc
    NI = C // P                # 128-channel blocks (8)
    NTB = T // P               # 128-token blocks (32)
    NCH = T // C               # 1024-token chunks (4)
    assert T == 4 * C and NI == 8

    x8, xb, wq8, wk8, wv, wo, scb, bo, y = handles

    x8_v = x8.ap().rearrange("(tb p) c -> p tb c", p=P)
    xb_v = xb.ap().rearrange("(tb p) c -> p tb c", p=P)
    wq8_v = wq8.ap().rearrange("(i p) c -> p i c", p=P)
    wk8_v = wk8.ap().rearrange("(i p) c -> p i c", p=P)
    wv_v = wv.ap().rearrange("(i p) c -> p i c", p=P)
    wo_v = wo.ap().rearrange("(i p) c -> p i c", p=P)

    Sqrt = mybir.ActivationFunctionType.Sqrt
    Exp = mybir.ActivationFunctionType.Exp
    AX = mybir.AxisListType.X
    MUL = mybir.AluOpType.mult
    ADD = mybir.AluOpType.add

    with ExitStack() as ctx:
        ctx.enter_context(nc.allow_low_precision(
            reason="fp8/bf16 matmul operands are intended; accum stays f32"))
        pers = ctx.enter_context(tc.tile_pool(name="pers", bufs=1))
        pw = ctx.enter_context(tc.tile_pool(name="pw", bufs=1))
        pa0s = ctx.enter_context(tc.tile_pool(name="pa0s", bufs=2))
        pprod = ctx.enter_context(tc.tile_pool(name="pprod", bufs=2))
        ptsc = ctx.enter_context(tc.tile_pool(name="ptsc", bufs=4))
        pxb = ctx.enter_context(tc.tile_pool(name="pxb", bufs=2))

        # --- persistent small tiles -------------------------------------
        # warmup operands first so the HAM warmup matmuls can issue asap
        warm_bf = pers.tile([P, 512], bf16, tag="warm_bf")
        nc.vector.memset(warm_bf, 0.0)
        warm_lhs = pers.tile([P, P], bf16, tag="warm_lhs")
        nc.vector.memset(warm_lhs, 0.0)
        epsq = pers.tile([P, 1], f32, tag="epsq")
        nc.vector.memset(epsq, EPS * EPS)
        ones_bf = pers.tile([P, 2], bf16, tag="ones_bf")
        nc.vector.memset(ones_bf, 1.0)
        onesr_f = pers.tile([1, P], f32, tag="onesr_f")
        nc.vector.memset(onesr_f, 1.0)
        onesr = pers.tile([1, P], f32r, tag="onesr")
        nc.vector.tensor_copy(out=onesr, in_=onesr_f)
        ident = pers.tile([P, P], f32, tag="ident")
        make_identity(nc, ident)
        scb_sb = pers.tile([1, C], f32r, tag="scb")
        bob = pers.tile([P, C], f32, tag="bob")
        rq = pers.tile([1, C], f32r, tag="rq")
        rks = pers.tile([1, C], f32r, tag="rks")
        rkt = pers.tile([P, 4 * NI], f32, tag="rkt")
        rqb = pers.tile([P, C], f32, tag="rqb")
        gsc32 = pers.tile([P, 1], f32, tag="gsc32")
        nc.vector.memset(gsc32, G_SCALE)
        gsc8 = pers.tile([P, 1], f32, tag="gsc8")
        nc.vector.memset(gsc8, GW_SCALE)
        pt_tiles = []
        for p in range(NI):
            pt = pers.tile([P, P], bf16, tag=f"pt{p}", name=f"pt_{p}")
            nc.vector.memset(pt, 0.0)
            pt_tiles.append(pt)
        a0s_all = pers.tile([P, NI * P], f32, tag="a0sall")
        sm16 = pers.tile([P, 2 * NI], f32, tag="sm16")

        # --- input DMAs issue first so the queues start moving ----------
        px8_stack = ExitStack()
        px8 = px8_stack.enter_context(tc.tile_pool(name="px8", bufs=1))
        x8_t = []
        for t in range(NTB // 2):
            xt8 = px8.tile([P, 2, C], f8, tag=f"x8_{t}", name=f"x8_{t}")
            for u in range(2):
                eng = nc.sync if u == 0 else nc.gpsimd
                eng.dma_start(out=xt8[:, u, :], in_=x8_v[:, 2 * t + u, :])
            x8_t.append(xt8)

        nc.sync.dma_start(
            out=scb_sb, in_=bass.AP(scb, 0, [[0, 1], [1, C]]).bitcast(f32r))
        nc.sync.dma_start(out=bob, in_=bass.AP(bo, 0, [[0, P], [1, C]]))

        # --- weight / data SBUF tiles -----------------------------------
        wq8_sb = pw.tile([P, NI, C], f8, tag="wq8")
        wk8_sb = pw.tile([P, NI, C], f8, tag="wk8")
        wv_sb = pw.tile([P, NI, C], bf16, tag="wv")
        wo_sb = pw.tile([P, NI, C], bf16, tag="wo")
        g8_sb = pw.tile([P, NI, C], f8, tag="g8")
        gwk_sb = pw.tile([P, NI, C], f8, tag="gwk")
        mt_sb = pw.tile([P, NI, C], bf16, tag="mt")

        # --- phase G: G = X^T X upper triangle, fp8 DoubleRow -----------
        # column tiles per ci-block-row: ci<4 -> [ci*128,512) + [512,1024);
        # ci>=4 -> [ci*128,1024). Lower triangle restored by PE transpose.
        def g_tiles(ci):
            if ci < 4:
                out = []
                if 512 - ci * P > 0:
                    out.append((ci * P, 512 - ci * P))
                out.append((512, 512))
                return out
            return [(ci * P, C - ci * P)]

        for i in range(NI):
            nc.scalar.dma_start(out=wk8_sb[:, i, :], in_=wk8_v[:, i, :])
        for i in range(NI):
            nc.scalar.dma_start(out=wq8_sb[:, i, :], in_=wq8_v[:, i, :])
        for i in range(NI):
            nc.gpsimd.dma_start(out=wo_sb[:, i, :], in_=wo_v[:, i, :])
        for i in range(NI):
            nc.gpsimd.dma_start(out=wv_sb[:, i, :], in_=wv_v[:, i, :])

        # lower-triangle restores; 22 with cj<4 interleave into pass B
        trans_a = [(i, j) for j in range(4) for i in range(j + 1, NI)]
        trans_b = [(i, j) for j in range(4, NI) for i in range(j + 1, NI)]
        tcnt = [0]

        with ExitStack() as ctxg:
            ppg = ctxg.enter_context(
                tc.tile_pool(name="ppg", bufs=1, space="PSUM"))

            def emit_trans_cast(ci, cj):
                ssc = ptsc.tile([P, P], f32, tag="tsc", name=f"tsc_{ci}_{cj}")
                nc.scalar.copy(
                    out=ssc, in_=g8_sb[:, cj, ci * P:(ci + 1) * P])
                return ssc

            def emit_trans(ci, cj, ssc):
                tp = ppg.tile([P, 512], f32, tag=f"g{tcnt[0] % 4}1",
                              name=f"gtp_{ci}_{cj}")
                nc.tensor.transpose(tp[:, 0:P], ssc, ident)
                nc.vector.tensor_copy(
                    out=g8_sb[:, ci, cj * P:(cj + 1) * P], in_=tp[:, 0:P])
                tcnt[0] += 1

            # HAM warmup: garbage matmuls on zeroed tiles while DMAs land
            warm_ps = ppg.tile([P, 512], f32, tag="g00", name="warm_ps")
            for i in range(N_WARM):
                nc.tensor.matmul(
                    warm_ps, warm_lhs, warm_bf,
                    start=(i == 0), stop=(i == N_WARM - 1))

            for ph in range(2):
                cis = range(4 * ph, 4 * ph + 4)
                pg = {}
                for ci in cis:
                    for ti, (o, w) in enumerate(g_tiles(ci)):
                        pg[(ci, ti)] = ppg.tile(
                            [P, w], f32, tag=f"g{ci % 4}{ti}",
                            name=f"g_{ci}_{ti}")
                pend = []
                ta = iter(trans_a)
                for t in range(NTB // 2):
                    for ci in cis:
                        for ti, (o, w) in enumerate(g_tiles(ci)):
                            nc.tensor.matmul(
                                pg[(ci, ti)],
                                x8_t[t][:, :, ci * P:(ci + 1) * P],
                                x8_t[t][:, :, o:o + w],
                                start=(t == 0), stop=(t == NTB // 2 - 1),
                                perf_mode=DR)
                    if ph == 1:
                        for _ in range(2):
                            nxt = next(ta, None)
                            if nxt is not None:
                                pend.append((nxt, emit_trans_cast(*nxt)))
                        while len(pend) > 2:
                            (ci_, cj_), ssc = pend.pop(0)
                            emit_trans(ci_, cj_, ssc)
                for ti in (0, 1):
                    for ci in cis:
                        tiles = g_tiles(ci)
                        if ti >= len(tiles):
                            continue
                        o, w = tiles[ti]
                        if ci % 2 == 0:
                            nc.vector.tensor_scalar_mul(
                                out=g8_sb[:, ci, o:o + w],
                                in0=pg[(ci, ti)], scalar1=gsc32)
                        else:
                            nc.scalar.mul(
                                out=g8_sb[:, ci, o:o + w],
                                in_=pg[(ci, ti)], mul=G_SCALE)

            # remaining lower-triangle blocks (need pass-B rows)
            for (ci_, cj_), ssc in pend:
                emit_trans(ci_, cj_, ssc)
            pend_b = [((ci, cj), emit_trans_cast(ci, cj))
                      for ci, cj in trans_b]
            for (ci_, cj_), ssc in pend_b:
                emit_trans(ci_, cj_, ssc)

        px8_stack.close()
        # px8/ppg freed; z pool and working psums reuse their space
        pz = ctx.enter_context(tc.tile_pool(name="pz", bufs=2))
        ppw = ctx.enter_context(tc.tile_pool(name="ppw", bufs=4, space="PSUM"))
        pps = ctx.enter_context(tc.tile_pool(name="pps", bufs=1, space="PSUM"))

        # a0 staging: psums recycle immediately into bf16 SBUF
        a0_bf = pers.tile([P, NI * P], bf16, tag="a0bf")
        ss_tiles = {}
        for ti, tname in enumerate(("q", "k")):
            for ci in range(2):
                ss_tiles[(tname, ci)] = pps.tile(
                    [2, 512], f32, tag=f"ps{2 * ti + ci}",
                    name=f"ss_{tname}_{ci}")

        # --- phase GW: GWk = G Wk^T, GWq = G Wq^T (fp8 DoubleRow) -------
        # norm products read the GW psum directly; their reduction matmuls
        # lag one ci behind the GW matmuls so the PE never waits on DVE.
        # Only GWk is materialized (fp8, for A0).
        def emit_gw_ci(w8_sb, nm, ci, keep):
            prod = pprod.tile([P, C], bf16, tag="prod")
            for half in range(2):
                ps = ppw.tile([P, 512], f32, tag="mm",
                              name=f"gw_{nm}_{ci}_{half}")
                for j in range(NI // 2):
                    nc.tensor.matmul(
                        ps,
                        g8_sb[:, 2 * j:2 * j + 2, ci * P:(ci + 1) * P],
                        w8_sb[:, 2 * j:2 * j + 2,
                              half * 512:(half + 1) * 512],
                        start=(j == 0), stop=(j == NI // 2 - 1),
                        perf_mode=DR)
                if keep is not None:
                    nc.scalar.mul(
                        out=keep[:, ci, half * 512:(half + 1) * 512],
                        in_=ps, mul=GW_SCALE)  # scalar free by GW time
                nc.vector.tensor_tensor(
                    out=prod[:, half * 512:(half + 1) * 512], in0=ps,
                    in1=w8_sb[:, ci, half * 512:(half + 1) * 512], op=MUL)
            return prod

        def emit_ss_ci(tname, prod, ci):
            for half in range(2):
                nc.tensor.matmul(
                    ss_tiles[(tname, half)],
                    ones_bf,
                    prod[:, half * 512:(half + 1) * 512],
                    start=(ci == 0), stop=(ci == NI - 1))

        # --- A0 head-pair blocks: A0[d,e] = sum_c GWk[c,d] Wq^T[c,e] ----
        def emit_a0():
            for quad in range(2):
                a0t = ppw.tile([P, 4 * P], f32, tag="mm", name=f"a0_{quad}")
                for pq in range(4):
                    p = quad * 4 + pq
                    for j in range(NI // 2):
                        nc.tensor.matmul(
                            a0t[:, pq * P:(pq + 1) * P],
                            gwk_sb[:, 2 * j:2 * j + 2, p * P:(p + 1) * P],
                            wq8_sb[:, 2 * j:2 * j + 2, p * P:(p + 1) * P],
                            start=(j == 0), stop=(j == NI // 2 - 1),
                            perf_mode=DR)
                nc.vector.tensor_copy(
                    out=a0_bf[:, quad * 512:(quad + 1) * 512], in_=a0t)

        # --- phase 1.5 (interleaved into Z0): norms+softmax+Pt ----------
        def emit_sqrt():
            for tname, dst in (("q", rq), ("k", rks)):
                for half in range(2):
                    nc.scalar.activation(
                        out=dst[0:1, half * 512:(half + 1) * 512],
                        in_=ss_tiles[(tname, half)][0:1, :], func=Sqrt,
                        bias=epsq[0:1, :])

        def emit_softmax_p1():
            rkt_ps = pps.tile([P, 512], f32, tag="ps0", name="rkt_ps")
            for i in range(NI):
                nc.tensor.matmul(
                    rkt_ps[:, 2 * i:2 * i + 2],
                    rks[0:1, i * P:(i + 1) * P],
                    onesr[0:1, 0:2],
                    start=(i == 0), stop=False)
            for i in range(NI):
                nc.tensor.matmul(
                    rkt_ps[:, 2 * (NI + i):2 * (NI + i) + 2],
                    scb_sb[0:1, i * P:(i + 1) * P],
                    onesr[0:1, 0:2],
                    start=False, stop=(i == NI - 1))
            nc.vector.tensor_copy(out=rkt, in_=rkt_ps[:, 0:4 * NI])
            nc.vector.reciprocal(
                out=rkt[:, 0:2 * NI], in_=rkt[:, 0:2 * NI])
            nc.vector.tensor_tensor(
                out=rkt[:, 0:2 * NI], in0=rkt[:, 0:2 * NI],
                in1=rkt[:, 2 * NI:4 * NI], op=MUL)

            for half in range(2):
                rqb_ps = pps.tile([P, 512], f32, tag="ps1", name="rqb_ps")
                nc.tensor.matmul(
                    rqb_ps, onesr,
                    rq[0:1, half * 512:(half + 1) * 512],
                    start=True, stop=True)
                nc.vector.reciprocal_approx_fast(
                    out=rqb[:, half * 512:(half + 1) * 512], in_=rqb_ps)

            for p in range(NI):
                nc.vector.tensor_scalar_mul(
                    out=a0s_all[:, p * P:(p + 1) * P],
                    in0=a0_bf[:, p * P:(p + 1) * P],
                    scalar1=rkt[:, 2 * p:2 * p + 1])
            nc.vector.tensor_tensor(
                out=a0s_all, in0=a0s_all, in1=rqb, op=MUL)
            # logits are normalized correlations * scale: |x| <= ~1.05,
            # exp never overflows, so no max-subtraction is needed
            nc.scalar.activation(out=a0s_all, in_=a0s_all, func=Exp)

        def emit_softmax_p2():
            for g in range(2 * NI):
                hs = slice((g % 2) * 64, (g % 2) * 64 + 64)
                nc.vector.reduce_sum(
                    out=sm16[hs, g:g + 1],
                    in_=a0s_all[hs, 64 * g:64 * (g + 1)], axis=AX)
            nc.vector.reciprocal(out=sm16, in_=sm16)
            for g in range(2 * NI):
                hs = slice((g % 2) * 64, (g % 2) * 64 + 64)
                nc.vector.tensor_scalar_mul(
                    out=a0s_all[hs, 64 * g:64 * (g + 1)],
                    in0=a0s_all[hs, 64 * g:64 * (g + 1)],
                    scalar1=sm16[hs, g:g + 1])

        def emit_pt(p):
            tp_ps = ppw.tile([P, 512], f32, tag="mm", name=f"tp_ps_{p}")
            nc.tensor.transpose(
                tp_ps[:, 0:P], a0s_all[:, p * P:(p + 1) * P], ident)
            nc.vector.tensor_copy(
                out=pt_tiles[p][0:64, 0:64], in_=tp_ps[0:64, 0:64])
            nc.vector.tensor_copy(
                out=pt_tiles[p][64:P, 64:P], in_=tp_ps[64:P, 64:P])

        # --- phase M^T: M^T[c, row] = sum_e Wv[row-pair e, c] P^T[e, d] -
        def emit_mt():
            for cb in range(NI):
                for quad in range(2):
                    ps = ppw.tile([P, 512], f32, tag="mm",
                                  name=f"mt_{cb}_{quad}")
                    for pq in range(4):
                        pr = quad * 4 + pq
                        nc.tensor.matmul(
                            ps[:, pq * P:(pq + 1) * P],
                            wv_sb[:, pr, cb * P:(cb + 1) * P],
                            pt_tiles[pr],
                            start=True, stop=True)
                    nc.vector.tensor_copy(
                        out=mt_sb[:, cb, quad * 512:(quad + 1) * 512], in_=ps)

        # --- phase 2: Z_ch = X_ch^T Wo^T ; Y_ch = (M^T)^T Z_ch ----------
        def z_alloc(ch):
            xbt = pxb.tile([P, NI, C], bf16, tag="xbt", name=f"xb_{ch}")
            for jb in range(NI):
                nc.sync.dma_start(
                    out=xbt[:, jb, :], in_=xb_v[:, ch * NI + jb, :])
            z_sb = pz.tile([P, NI, C], bf16, tag="z", name=f"z_{ch}")
            return xbt, z_sb

        def z_group(xbt, z_sb, ch, cb, half):
            zps = ppw.tile([P, 512], f32, tag="mm",
                           name=f"z_{ch}_{cb}_{half}")
            for jb in range(NI):
                nc.tensor.matmul(
                    zps,
                    xbt[:, jb, cb * P:(cb + 1) * P],
                    wo_sb[:, jb, half * 512:(half + 1) * 512],
                    start=(jb == 0), stop=(jb == NI - 1))
            nc.vector.tensor_copy(
                out=z_sb[:, cb, half * 512:(half + 1) * 512], in_=zps)

        def emit_z(ch, hooks=None):
            xbt, z_sb = z_alloc(ch)
            for grp in range(2 * NI):
                z_group(xbt, z_sb, ch, grp // 2, grp % 2)
                if hooks and grp in hooks:
                    hooks[grp]()
            return z_sb

        def emit_y(ch, z_sb):
            for rb in range(NI):
                for half in range(2):
                    yps = ppw.tile([P, 512], f32, tag="mm",
                                   name=f"y_{ch}_{rb}_{half}")
                    for cb in range(NI):
                        nc.tensor.matmul(
                            yps,
                            mt_sb[:, cb, rb * P:(rb + 1) * P],
                            z_sb[:, cb, half * 512:(half + 1) * 512],
                            start=(cb == 0), stop=(cb == NI - 1))
                    ysb = pa0s.tile([P, 512], f32, tag="ysb")
                    nc.vector.tensor_tensor(
                        out=ysb, in0=yps,
                        in1=bob[:, half * 512:(half + 1) * 512], op=ADD)
                    base = (512 * rb + ch) * C + half * 512
                    if ch == NCH - 1:
                        # last chunk: split by partition range across all
                        # three DMA queues so the final drain parallelizes
                        for qi, (p0, p1) in enumerate(
                                ((0, 43), (43, 86), (86, P))):
                            yeng = (nc.gpsimd, nc.sync, nc.scalar)[qi]
                            yeng.dma_start(
                                out=bass.AP(y, base + p0 * 4 * C,
                                            [[4 * C, p1 - p0], [1, 512]]),
                                in_=ysb[p0:p1, :])
                    else:
                        yeng = (nc.gpsimd, nc.sync, nc.scalar)[
                            (2 * rb + half) % 3]
                        yeng.dma_start(
                            out=bass.AP(y, base, [[4 * C, P], [1, 512]]),
                            in_=ysb)

        # phase-1 tail: GW (with lagged ss) interleaved 2:1 with Z0 groups
        xbt0, z0 = z_alloc(0)
        z0_grp = [0]

        def z0_step(n=1):
            for _ in range(n):
                if z0_grp[0] < 2 * NI:
                    z_group(xbt0, z0, 0, z0_grp[0] // 2, z0_grp[0] % 2)
                    z0_grp[0] += 1

        for w8_sb, keep, nm in ((wk8_sb, gwk_sb, "k"), (wq8_sb, None, "q")):
            prev = None
            for ci in range(NI):
                prod = emit_gw_ci(w8_sb, nm, ci, keep)
                if prev is not None:
                    emit_ss_ci(nm, prev, ci - 1)
                prev = prod
                if ci % 2 == 1:
                    z0_step()
            emit_ss_ci(nm, prev, NI - 1)

        emit_sqrt()
        emit_a0()
        z0_step(2)
        emit_softmax_p1()
        z0_step(4)
        emit_softmax_p2()
        z0_step(16)

        pt_hooks = {}
        for p in range(NI):
            pt_hooks[7 + p] = (lambda pp: lambda: emit_pt(pp))(p)
        z1 = emit_z(1, pt_hooks)
        emit_mt()
        emit_y(0, z0)
        z2 = emit_z(2)
        emit_y(1, z1)
        z3 = emit_z(3)
        emit_y(2, z2)
        emit_y(3, z3)


def build_nc(C=C_FULL, T=T_FULL):
    nc = bacc.Bacc("TRN2", target_bir_lowering=False)
    x8 = nc.dram_tensor("x8", [T, C], f8, kind="ExternalInput")
    xb = nc.dram_tensor("xb", [T, C], bf16, kind="ExternalInput")
    wq8 = nc.dram_tensor("wq8", [C, C], f8, kind="ExternalInput")
    wk8 = nc.dram_tensor("wk8", [C, C], f8, kind="ExternalInput")
    wv = nc.dram_tensor("wv", [C, C], bf16, kind="ExternalInput")
    wo = nc.dram_tensor("wo", [C, C], bf16, kind="ExternalInput")
    scb = nc.dram_tensor("scb", [C], f32, kind="ExternalInput")
    bo = nc.dram_tensor("bo", [C], f32, kind="ExternalInput")
    y = nc.dram_tensor("y", [T, C], f32, kind="ExternalOutput")
    with tile.TileContext(nc) as tc:
        emit_kernel(tc, (x8, xb, wq8, wk8, wv, wo, scb, bo, y), C, T)
    nc.compile()
    return nc


def make_in_maps(x, Wq, Wk, Wv, scale, Wo, bo, C=C_FULL, T=T_FULL):
    """Host-side prep: fp8/bf16 casts, transposes, per-channel scale."""
    import ml_dtypes
    f = np.float32
    e4 = ml_dtypes.float8_e4m3
    b16 = ml_dtypes.bfloat16
    wq8 = np.ascontiguousarray(
        (np.asarray(Wq, dtype=f).T * W_SCALE)).astype(e4)
    wk8 = np.ascontiguousarray(
        (np.asarray(Wk, dtype=f).T * W_SCALE)).astype(e4)
    wv_b = np.ascontiguousarray(np.asarray(Wv, dtype=f)).astype(b16)
    wo_b = np.ascontiguousarray(np.asarray(Wo, dtype=f).T).astype(b16)
    # the 8x on device-side ss (psum-sourced norm products carry G/2 * W*16
    # scaling) cancels via logits * (scb*8) / (rks_meas * rq_meas)
    scb = np.ascontiguousarray(
        np.repeat(np.asarray(scale, dtype=f).reshape(-1), 64) * 8.0)
    bo_h = np.ascontiguousarray(np.asarray(bo, dtype=f).reshape(-1))
    x = np.asarray(x, dtype=f)
    in_maps = []
    for b in range(x.shape[0]):
        in_maps.append({
            "x8": x[b].astype(e4),
            "xb": x[b].astype(b16),
            "wq8": wq8, "wk8": wk8, "wv": wv_b, "wo": wo_b,
            "scb": scb, "bo": bo_h,
        })
    return in_maps


_NC_CACHE = {}


def kernel(x, Wq, Wk, Wv, scale, Wo, bo, trace=False, **run_kwargs):
    from concourse.bass_utils import run_bass_kernel_spmd

    key = (C_FULL, T_FULL)
    if key not in _NC_CACHE:
        _NC_CACHE[key] = build_nc(*key)
    nc = _NC_CACHE[key]
    in_maps = make_in_maps(x, Wq, Wk, Wv, scale, Wo, bo)
    res = run_bass_kernel_spmd(
        nc, in_maps, core_ids=list(range(len(in_maps))),
        trace=trace, **run_kwargs)
    out = np.stack([r["y"] for r in res.results])
    kernel.last_results = res
    return out
